# revision 93
# baseline (speedup 1.0000x reference)
"""CBTree (levelized complete 4-ary tree, depth 8, D=512) forward on 8 TRN2 NeuronCores.

Math: per level,  h = tanh(x + sum_b lc[b]*(h_b @ wl.T) + rc[b]*(h_b @ wr.T)).
By linearity the per-child matmuls collapse to two matmuls of weighted child
sums  agg = Ul' @ (wl.T/3) + Ur' @ (wr.T/3),  Ul' = 3h0 + 2h1 + h2,
Ur' = h1 + 2h2 + 3h3  (4x FLOP reduction, 32 PE cycles/parent at f16).

Sharding: the 16 level-2 subtrees are sharded 2-per-core (b-major order);
each core runs global levels 7..2 of its forest locally, then contributes
its two scaled+transformed level-2 hiddens to a 4KB AllReduce(add) that
yields the level-1 aggregates, and every core redundantly finishes the tiny
levels 1+0, all feature-major ([128 partitions, 4 d-tiles, n] with base-4
digit-reversed node storage so each child block is a contiguous col range).

Perf structure (cost-model timeline driven):
 - The leaf-level weighted sums (75% of DVE work and half the leaf HBM
   bytes) are precomputed on the HOST: the kernel streams packed
   [Ul6|Ur6|x5] records, one DMA per 256-col chunk.
 - PSUM accumulation groups are bank-granular: every concurrent group owns
   a full [128, 512] bank. Big chunks use 4 per-et tiles (precise Tile
   deps -> per-et x-add/tanh pipeline against the matmul stream); latency
   critical spots use one bank with a single start..stop group.
 - Matmuls are emitted kt0-3 all-ets then kt4-7 et-outer, so Ur has a full
   PE pass of slack and each et's accumulation stops early.
 - x-adds/tanhs are emitted DEFERRED (flushed just before the consumer
   level's U) so they never sit in front of U work on the DVE/Act queues.
 - x enters via DVE adds on throughput chunks, and via identity-weight PE
   injection on the latency-critical ones.
 - The tiny levels (w <= 32) and the root skip the U stage entirely:
   pre-combined weights Vb = lc_b*wl.T + rc_b*wr.T consume child h
   directly (2x trivial PE columns, no DVE hop).
"""
import sys

import numpy as np

sys.path.insert(0, "/opt/trn_rl_repo")

import concourse.bass as bass  # noqa: E402,F401
import concourse.bacc as bacc  # noqa: E402
import concourse.tile as tile  # noqa: E402
from concourse import mybir  # noqa: E402
from concourse.bass_utils import run_bass_kernel_spmd  # noqa: E402

NCORES = 8
D = 512
NT = 4  # d-tiles of 128
B = 4
DEPTH = 8
# local levels L=0..6 <-> global levels 2..8 ; per-core node counts
NLOC = [2 * 4**l for l in range(7)]  # [2, 8, 32, 128, 512, 2048, 8192]

PRECISION = "f16"  # storage/matmul dtype of the pipeline (test.py prints it)

# debug: dump an intermediate h level to a "dbg" output (None to disable)
DEBUG_DUMP = None  # e.g. ("h", 5) local level; DEBUG_DUMP_N = its width
DEBUG_DUMP_N = 0
INJECT_ALL = False  # debug: PE-inject x on every chunk (no DVE x-add)

FP32 = mybir.dt.float32
F16 = mybir.dt.float16

# L5 (global level 7) chunking: two 128-wide starter chunks for fast
# pipeline fill, then 256-wide. "evens" cover the first half of each of the
# four 512-wide child blocks consumed by L4 chunks 0+1, "odds" the rest.
L5_CHUNKS = [(0, 128), (128, 128), (256, 256), (512, 256), (768, 256),
             (1024, 256), (1280, 256), (1536, 256), (1792, 256)]
L5_EVENS = [0, 1, 3, 5, 7]
L5_ODDS = [2, 4, 6, 8]
# L4 in 256-wide chunks: chunk 0 consumes the even L5 chunks, chunk 1 the
# odd ones (child block b of cols [c0,c0+256) is h5[b*512+c0 : +256))
L4_CHUNKS = [(0, 256), (256, 256)]
# x storage offsets for local levels 4..0 inside the packed xrest tensor
XR_OFF = {4: 0, 3: 512, 2: 640, 1: 672, 0: 680}
XR_TOT = 682
# record offsets for the [Ul(w) | Ur(w) | x5(w)] chunk records
REC_OFF = []
_o = 0
for _c0, _w in L5_CHUNKS:
    REC_OFF.append(_o)
    _o += 3 * _w
REC_TOT = _o  # 6144


# ---------------------------------------------------------------- host helpers
def _rev4(idx: np.ndarray, ndig: int) -> np.ndarray:
    r = np.zeros_like(idx)
    q = idx.copy()
    for _ in range(ndig):
        r = (r << 2) | (q & 3)
        q >>= 2
    return r


def _storage_nodes(level: int) -> np.ndarray:
    """Local node index stored at each storage column of local level ``level``.

    Local node j = r*4^L + q (r = which of the core's two subtrees) is stored
    at column 2*rev4(q) + r; returns the inverse map.
    """
    n = NLOC[level]
    j = np.arange(n, dtype=np.int64)
    r, q = j >> (2 * level), j & ((1 << (2 * level)) - 1)
    pos = 2 * _rev4(q, level) + r
    inv = np.empty(n, dtype=np.int64)
    inv[pos] = j
    return inv


# ---------------------------------------------------------------- device build
def _build_nc(with_tail=True):
    nc = bacc.Bacc(
        "TRN2", target_bir_lowering=False, debug=False, num_devices=NCORES
    )

    w2_d = nc.dram_tensor("w2", [2 * D, D], F16, kind="ExternalInput")
    u6x5_d = nc.dram_tensor("u6x5", [D, REC_TOT], F16, kind="ExternalInput")
    xrest_d = nc.dram_tensor("xrest", [D, XR_TOT], F16, kind="ExternalInput")
    xt1_d = nc.dram_tensor("xt1", [128, 16], F16, kind="ExternalInput")
    xt0_d = nc.dram_tensor("xt0", [128, NT], F16, kind="ExternalInput")
    # per-core level-1 coefficient masks: cols 0:4 = (3-b) at the core's
    # two parent columns (0 elsewhere), cols 4:8 = b likewise
    clcr_d = nc.dram_tensor("clcr", [128, 32], F16, kind="ExternalInput")
    # per-child-slot combined weights Vb = lc_b*wl.T + rc_b*wr.T, used by
    # the tiny levels + root to consume child h directly (no DVE U stage)
    vw_d = nc.dram_tensor("vw", [16 * 128, D], F16, kind="ExternalInput")
    idm_d = nc.dram_tensor("identm", [128, 128], F16, kind="ExternalInput")
    out_d = nc.dram_tensor("out", [1, D], FP32, kind="ExternalOutput")
    dbg_d = (nc.dram_tensor("dbg", [D, DEBUG_DUMP_N], F16,
                            kind="ExternalOutput")
             if DEBUG_DUMP_N else None)

    def fm_ap(handle):
        # DRAM [512, n] -> [128p, 4t, n] with feature d = t*128 + p
        return handle.ap().rearrange("(t p) n -> p t n", p=128)

    mult, add = mybir.AluOpType.mult, mybir.AluOpType.add
    Tanh = mybir.ActivationFunctionType.Tanh

    with tile.TileContext(nc) as tc:
        with (
            tc.tile_pool(name="const", bufs=1) as const,
            tc.tile_pool(name="hp", bufs=1) as hp,
            tc.tile_pool(name="recp", bufs=3) as recp,
            tc.tile_pool(name="up", bufs=2) as up,
            tc.tile_pool(name="tmpp", bufs=2) as tmpp,
            tc.tile_pool(name="psum", bufs=8, space="PSUM") as psum,
            tc.tile_pool(name="dram", bufs=1, space="DRAM") as dram,
        ):
            # --- constants; w2 in two halves so kt0-3 lands before chunk 0
            w2_sb = const.tile([128, 8, D], F16, tag="w2", name="w2sb")
            w2ap = w2_d.ap().rearrange("(kt p) e -> p kt e", p=128)
            nc.sync.dma_start(out=w2_sb[:, 0:4, :], in_=w2ap[:, 0:4, :])

            late = {}  # extra const DMAs interleaved into the record stream
            late[0] = [(lambda: nc.sync.dma_start(
                out=w2_sb[:, 4:8, :], in_=w2ap[:, 4:8, :]))]
            xr_sb = const.tile([128, NT, XR_TOT], F16, tag="xr", name="xrsb")
            late[2] = [(lambda: nc.sync.dma_start(
                out=xr_sb[:], in_=fm_ap(xrest_d)))]
            xt1_sb = const.tile([128, NT, 4], F16, tag="xt1", name="xt1sb")
            xt0_sb = const.tile([128, NT], F16, tag="xt0", name="xt0sb")
            idm_sb = const.tile([128, 128], F16, tag="idm", name="idmsb")
            clcr_sb = const.tile([128, NT, 8], F16, tag="clcr", name="clcrsb")
            vw_sb = const.tile([128, 16, D], F16, tag="vw", name="vwsb")
            vwap = vw_d.ap().rearrange("(bk p) e -> p bk e", p=128)
            late[6] = [(lambda: nc.sync.dma_start(out=vw_sb[:], in_=vwap))]
            late[4] = [
                (lambda: nc.sync.dma_start(out=xt1_sb[:], in_=xt1_d.ap())),
                (lambda: nc.sync.dma_start(out=xt0_sb[:], in_=xt0_d.ap())),
                (lambda: nc.sync.dma_start(out=idm_sb[:], in_=idm_d.ap())),
                (lambda: nc.sync.dma_start(out=clcr_sb[:], in_=clcr_d.ap())),
            ]

            # h storage: per level, per chunk index -> (col0, width, tile)
            h_tiles = {L: [] for L in range(6)}

            def h_pieces(L, cols):
                """[(off, width, AP)] covering level-L h storage col range,
                possibly spanning several chunk tiles."""
                c0, c1 = cols
                out = []
                pos = c0
                while pos < c1:
                    for t0, w, t in h_tiles[L]:
                        if t0 <= pos < t0 + w:
                            e = min(c1, t0 + w)
                            out.append((pos - c0, e - pos,
                                        t[:, :, pos - t0:e - t0]))
                            pos = e
                            break
                    else:
                        raise AssertionError(f"no tile for L{L} col {pos}")
                return out

            def h_slice_kt(L, cols, kt):
                """AP [128, len(cols)] of one kt-plane of level-L h."""
                c0, c1 = cols
                for t0, w, t in h_tiles[L]:
                    if t0 <= c0 and c1 <= t0 + w:
                        return t[:, kt, c0 - t0:c1 - t0]
                raise AssertionError(f"no tile covers L{L} cols {cols}")

            def h_slice(L, cols):
                """AP [128, NT, len(cols)] of level-L h at storage col range
                (must be covered by a single chunk tile)."""
                p = h_pieces(L, cols)
                assert len(p) == 1, f"L{L} cols {cols} spans {len(p)} tiles"
                return p[0][2]

            def weighted_sums(L, c0, w, per_t):
                """Ul = 3*H0 + 2*H1 + H2 ; Ur = H1 + 2*H2 + 3*H3 from level
                L+1 children. per_t: emit per-t-slice ops (small levels) so
                each t lands as soon as its source tanh is done."""
                n1 = NLOC[L]
                Ul = up.tile([128, NT, w], F16, tag="Ul", name="Ul")
                Ur = up.tile([128, NT, w], F16, tag="Ur", name="Ur")
                tA = tmpp.tile([128, NT, w], F16, tag="tA", name="tA")
                tB = tmpp.tile([128, NT, w], F16, tag="tB", name="tB")

                def cols(b):
                    return (b * n1 + c0, b * n1 + c0 + w)

                if not per_t:
                    # ts at 4x + tt at 2x: best DVE throughput (1.5c per U);
                    # ops split per covering source tile where needed
                    HP = [h_pieces(L + 1, cols(b)) for b in range(B)]

                    def ts(dst, b, s):
                        for off, wd, ap in HP[b]:
                            nc.vector.tensor_scalar_mul(
                                dst[:, :, off:off + wd], ap, s)

                    def tt(dst, src, b):
                        for off, wd, ap in HP[b]:
                            nc.vector.tensor_add(
                                dst[:, :, off:off + wd],
                                src[:, :, off:off + wd], ap)

                    ts(tA, 0, 3.0)
                    ts(tB, 1, 2.0)
                    nc.vector.tensor_add(tA[:], tA[:], tB[:])
                    tt(Ul, tA, 2)
                    ts(tA, 3, 3.0)
                    ts(tB, 2, 2.0)
                    nc.vector.tensor_add(tA[:], tA[:], tB[:])
                    tt(Ur, tA, 1)
                else:
                    # tiny levels: latency-bound, fewest serial ops wins
                    H = [h_slice(L + 1, cols(b)) for b in range(B)]
                    nc.vector.scalar_tensor_tensor(
                        out=tA[:], in0=H[0], scalar=3.0, in1=H[2],
                        op0=mult, op1=add)
                    nc.vector.scalar_tensor_tensor(
                        out=Ul[:], in0=H[1], scalar=2.0, in1=tA[:],
                        op0=mult, op1=add)
                    nc.vector.scalar_tensor_tensor(
                        out=tB[:], in0=H[3], scalar=3.0, in1=H[1],
                        op0=mult, op1=add)
                    nc.vector.scalar_tensor_tensor(
                        out=Ur[:], in0=H[2], scalar=2.0, in1=tB[:],
                        op0=mult, op1=add)
                return Ul, Ur

            def level_matmuls(ul_of_kt, ur_of_kt, x_flat, x_of_et, h_out,
                              h_out_t, w, inject=False, granular=False):
                """psum = W2.T @ [Ul;Ur]: kt0-3 for all ets (kt-outer so
                matmuls start after just Ul's first slice), then kt4-7
                et-outer so each et's accumulation STOPS early. One wide
                psum tile holds all four ets at w-wide slots, so the x-add
                and the tanh are a SINGLE DVE / Act op per chunk (per-op
                SEQ+sem overhead dominates at small widths).

                inject: add x on the PE via an identity-weight matmul
                (start=True) instead of the DVE - used on the latency
                critical chunks (level transitions, small levels) where
                the extra PE columns are cheaper than the DVE+sem hop.
                With inject the finish is act-only, and granular=True
                emits per-et acts (each starts right after its own
                accumulation stop; the consumer U sees the last slice
                ~1 act earlier than a single wide act).

                Returns the deferred finish closure (flushed later so this
                work never sits in front of the next chunk's U).

                PSUM accumulation groups are BANK-granular on HW, and the
                Tile dependency tracking is TILE-granular, so each et gets
                its own [128, 512] one-bank tile: the per-et finish ops
                pipeline against the matmul stream instead of serializing
                behind the last write to a shared tile.

                For 4w <= 512 the whole level fits ONE bank as a single
                accumulation group (start on the one inject matmul, stop
                only on the final matmul): one inject, one act - minimal
                serial-latency for the tiny levels."""
                if inject and 4 * w <= 512:
                    ps = psum.tile([128, 512], FP32, tag="agg", name="ps")
                    nc.tensor.matmul(ps[:, 0:4 * w], idm_sb[:], x_flat,
                                     start=True, stop=False)
                    for kt in range(4):
                        for et in range(NT):
                            nc.tensor.matmul(
                                ps[:, et * w:(et + 1) * w],
                                w2_sb[:, kt, et * 128:(et + 1) * 128],
                                ul_of_kt(kt), start=False, stop=False)
                    for et in range(NT):
                        for kt in range(4):
                            nc.tensor.matmul(
                                ps[:, et * w:(et + 1) * w],
                                w2_sb[:, 4 + kt, et * 128:(et + 1) * 128],
                                ur_of_kt(kt), start=False,
                                stop=(kt == 3 and et == NT - 1))

                    def finish():
                        nc.scalar.activation(out=h_out, in_=ps[:, 0:4 * w],
                                             func=Tanh)

                    return finish

                pss = [psum.tile([128, 512], FP32, tag="agg",
                                 name=f"ps{et}") for et in range(NT)]
                if inject:
                    for et in range(NT):
                        nc.tensor.matmul(
                            pss[et][:, 0:w], idm_sb[:],
                            x_of_et(et), start=True, stop=False)
                for kt in range(4):
                    for et in range(NT):
                        nc.tensor.matmul(
                            pss[et][:, 0:w],
                            w2_sb[:, kt, et * 128:(et + 1) * 128],
                            ul_of_kt(kt),
                            start=(kt == 0 and not inject), stop=False,
                        )
                for et in range(NT):
                    for kt in range(4):
                        nc.tensor.matmul(
                            pss[et][:, 0:w],
                            w2_sb[:, 4 + kt, et * 128:(et + 1) * 128],
                            ur_of_kt(kt),
                            start=False, stop=(kt == 3),
                        )

                def finish():
                    for et in range(NT):
                        if not inject:
                            nc.vector.tensor_add(
                                pss[et][:, 0:w], pss[et][:, 0:w],
                                x_of_et(et))
                        nc.scalar.activation(
                            out=h_out_t(et), in_=pss[et][:, 0:w], func=Tanh)

                return finish

            def level_direct(L, w, h_out):
                """Tiny levels: consume child h directly through the
                pre-combined Vb weights - 2x the (trivial) PE columns but
                no DVE weighted-sum stage on the critical path. Single
                bank, single accumulation group, single act."""
                n1 = NLOC[L]
                ps = psum.tile([128, 512], FP32, tag="agg", name="ps")
                xo = XR_OFF[L]
                nc.tensor.matmul(ps[:, 0:4 * w], idm_sb[:],
                                 xr_sb[:, :, xo:xo + w], start=True,
                                 stop=False)
                for b in range(B):
                    for kt in range(4):
                        hs = h_slice_kt(L + 1, (b * n1, b * n1 + w), kt)
                        for et in range(NT):
                            nc.tensor.matmul(
                                ps[:, et * w:(et + 1) * w],
                                vw_sb[:, b * 4 + kt,
                                      et * 128:(et + 1) * 128],
                                hs, start=False,
                                stop=(b == 3 and kt == 3 and et == NT - 1))
                nc.scalar.activation(out=h_out, in_=ps[:, 0:4 * w],
                                     func=Tanh)

            # ---------------- schedule: L5 evens, L4 0-1, L5 odds, L4 2-3,
            # then L3..L0, then the tail
            # L4 chunk 0 is slotted one odd chunk late so its U (which
            # only needs the evens) computes under that chunk's matmuls
            schedule = ([("L5", ci, 0) for ci in L5_EVENS]
                        + [("L5", L5_ODDS[0], 1), ("L4", 0, None)]
                        + [("L5", ci, 1) for ci in L5_ODDS[1:]]
                        + [("L4", 1, None)]
                        + [("S", 3, None), ("S", 2, None),
                           ("S", 1, None), ("S", 0, None)])

            # pending finish() closures per level, flushed just before the
            # first consumer U (so x-add/tanh never block the PE stream);
            # the last one flushes granular so the consumer's first U slice
            # starts as soon as the matching tanh lands
            pending = {}  # (level, group) -> [finish closures]

            def flush(L, grp=None):
                for key in [k for k in pending if k[0] == L
                            and (grp is None or k[1] == grp)]:
                    for fn in pending.pop(key):
                        fn()

            n_l5_seen = 0
            for kind, ci, grp in schedule:
                # cap deferral depth so psum-ring WAR releases (the tanh
                # reads) never trail the PE by more than the ring depth
                for key in list(pending):
                    while len(pending[key]) > 1:
                        pending[key].pop(0)()
                if kind == "L5":
                    c0, w = L5_CHUNKS[ci]
                    rec = recp.tile([128, NT, 3 * w], F16, tag="rec",
                                    name=f"rec{ci}")
                    nc.sync.dma_start(
                        out=rec[:],
                        in_=fm_ap(u6x5_d)[:, :, REC_OFF[ci]:REC_OFF[ci] + 3 * w],
                    )
                    if n_l5_seen in late:
                        for fn in late.pop(n_l5_seen):
                            fn()
                    n_l5_seen += 1
                    ht = hp.tile([128, NT, w], F16, tag=f"h5_{ci}",
                                 name=f"h5_{ci}")
                    h_tiles[5].append((c0, w, ht))
                    # the last TWO chunks of a group get inject+granular:
                    # their finishes are act-only, so the consumer's U and
                    # the psum-bank WAR release never queue behind a 3us
                    # tt/act chain on the DVE/Act engines
                    last_of_group = (ci == L5_EVENS[-1] or ci == L5_ODDS[-1]
                                     or INJECT_ALL)
                    fin = level_matmuls(
                        lambda kt, rec=rec, w=w: rec[:, kt, 0:w],
                        lambda kt, rec=rec, w=w: rec[:, kt, w:2 * w],
                        rec[:, :, 2 * w:3 * w],
                        lambda et, rec=rec, w=w: rec[:, et, 2 * w:3 * w],
                        ht[:],
                        lambda et, ht=ht: ht[:, et, :], w,
                        inject=last_of_group, granular=last_of_group,
                    )
                    pending.setdefault((5, grp), []).append(fin)
                else:
                    if kind == "L4":
                        L, (c0, w) = 4, L4_CHUNKS[ci]
                    else:
                        L = ci
                        c0, w = 0, NLOC[L]
                    if kind == "L4":
                        flush(5, ci)  # only the half this chunk consumes
                    else:
                        flush(L + 1)
                    if w <= 32:
                        ht = hp.tile([128, NT, w], F16, tag=f"h{L}_{c0}",
                                     name=f"h{L}_{c0}")
                        h_tiles[L].append((c0, w, ht))
                        level_direct(L, w, ht[:])
                        continue
                    Ul, Ur = weighted_sums(L, c0, w, False)
                    ht = hp.tile([128, NT, w], F16, tag=f"h{L}_{c0}",
                                 name=f"h{L}_{c0}")
                    h_tiles[L].append((c0, w, ht))
                    xo = XR_OFF[L] + c0
                    # everything from the last L4 chunk down sits on the
                    # serial critical path: PE-inject x, granular acts at
                    # the larger widths
                    last = (kind == "L4" and ci == 1) or kind == "S"
                    fin = level_matmuls(
                        lambda kt, Ul=Ul: Ul[:, kt, :],
                        lambda kt, Ur=Ur: Ur[:, kt, :],
                        xr_sb[:, :, xo:xo + w],
                        lambda et, xo=xo, w=w: xr_sb[:, et, xo:xo + w],
                        ht[:],
                        lambda et, ht=ht: ht[:, et, :], w,
                        inject=last, granular=last and w >= 128,
                    )
                    pending.setdefault((L, None), []).append(fin)
            flush(0)

            # ---------------- tail: AllGather the 16 level-2 hiddens
            # (node-major payload: single DMA each way), then levels 1+0
            # feature-major, redundantly on every core.
            if dbg_d is not None and DEBUG_DUMP[0] == "h":
                for t0, wdt, tt_ in h_tiles[DEBUG_DUMP[1]]:
                    nc.sync.dma_start(
                        out=dbg_d.ap()[:, t0:t0 + wdt].rearrange(
                            "(t p) n -> p t n", p=128),
                        in_=tt_[:],
                    )

            # Tail v3: each core computes its CONTRIBUTION to the
            # level-1 aggregates (its two h2 vectors scaled by host-built
            # per-core coefficient masks, pushed through W2), one
            # AllReduce(add) sums them across cores, and every core
            # redundantly finishes h1 + root. Single DMA each way.
            h2t = h_tiles[0][0][2]
            ul4 = up.tile([128, NT, 4], F16, tag="ul4", name="ul4")
            ur4 = up.tile([128, NT, 4], F16, tag="ur4", name="ur4")
            nc.vector.tensor_mul(ul4[:, :, 0:2], h2t[:], clcr_sb[:, :, 0:2])
            nc.vector.tensor_mul(ul4[:, :, 2:4], h2t[:], clcr_sb[:, :, 2:4])
            nc.vector.tensor_mul(ur4[:, :, 0:2], h2t[:], clcr_sb[:, :, 4:6])
            nc.vector.tensor_mul(ur4[:, :, 2:4], h2t[:], clcr_sb[:, :, 6:8])
            pst = psum.tile([128, 512], FP32, tag="agg", name="pst")
            for kt in range(4):
                for et in range(NT):
                    nc.tensor.matmul(
                        pst[:, et * 4:(et + 1) * 4],
                        w2_sb[:, kt, et * 128:(et + 1) * 128],
                        ul4[:, kt, :], start=(kt == 0 and et == 0),
                        stop=False)
            for kt in range(4):
                for et in range(NT):
                    nc.tensor.matmul(
                        pst[:, et * 4:(et + 1) * 4],
                        w2_sb[:, 4 + kt, et * 128:(et + 1) * 128],
                        ur4[:, kt, :], start=False,
                        stop=(kt == 3 and et == NT - 1))
            term_sb = const.tile([128, NT, 4], F16, tag="term", name="term")
            nc.scalar.copy(out=term_sb[:], in_=pst[:, 0:16])

            cc_in = dram.tile([D, 4], F16, tag="cc_in", name="cc_in")
            cc_out = dram.tile([D, 4], F16, tag="cc_out", name="cc_out")
            nc.sync.dma_start(
                out=cc_in[:, :].rearrange("(t p) n -> p t n", p=128),
                in_=term_sb[:],
            )
            if with_tail:
                nc.gpsimd.collective_compute(
                    "AllReduce",
                    mybir.AluOpType.add,
                    replica_groups=[list(range(NCORES))],
                    ins=[cc_in.opt()],
                    outs=[cc_out.opt()],
                )
            else:  # collective-free variant for single-core cost simulation
                nc.sync.dma_start(out=cc_out[:, :], in_=cc_in[:, :])
            # agg1 feature-major [128, t, parent]: single gather DMA
            G1 = const.tile([128, NT, 4], F16, tag="G1", name="G1")
            nc.sync.dma_start(
                out=G1[:],
                in_=cc_out[:, :].rearrange("(t p) n -> p t n", p=128),
            )

            # h1 = tanh(x1 + agg1): both added on the idle PE via identity
            # injects (single accumulation group in one bank), one act
            ps1 = psum.tile([128, 512], FP32, tag="agg", name="ps1")
            nc.tensor.matmul(ps1[:, 0:16], idm_sb[:], xt1_sb[:, :, :],
                             start=True, stop=False)
            nc.tensor.matmul(ps1[:, 0:16], idm_sb[:], G1[:],
                             start=False, stop=True)
            h1 = const.tile([128, NT, 4], F16, tag="h1", name="h1")
            nc.scalar.activation(out=h1[:], in_=ps1[:, 0:16], func=Tanh)

            # root: consume h1 directly through the Vb weights
            ps0 = psum.tile([128, 512], FP32, tag="agg", name="ps0")
            nc.tensor.matmul(ps0[:, 0:4], idm_sb[:], xt0_sb[:],
                             start=True, stop=False)
            for b in range(B):
                for kt in range(4):
                    for et in range(NT):
                        nc.tensor.matmul(
                            ps0[:, et:et + 1],
                            vw_sb[:, b * 4 + kt, et * 128:(et + 1) * 128],
                            h1[:, kt, b:b + 1], start=False,
                            stop=(b == 3 and kt == 3 and et == NT - 1))
            root_sb = const.tile([128, NT], FP32, tag="root", name="rootsb")
            nc.scalar.activation(out=root_sb[:], in_=ps0[:, 0:4], func=Tanh)
            nc.sync.dma_start(
                out=out_d.ap().rearrange("o (t p) -> p (o t)", p=128),
                in_=root_sb[:],
            )

    nc.compile()
    return nc


_NC_CACHE = {}


def _get_nc():
    key = PRECISION
    if key not in _NC_CACHE:
        _NC_CACHE[key] = _build_nc()
    return _NC_CACHE[key]


# ---------------------------------------------------------------- entry point
def kernel(vectors, wl, wr, branching, depth):
    out, _ = _run(vectors, wl, wr, branching, depth, trace=False)
    return out


def _run(vectors, wl, wr, branching, depth, trace=False):
    assert int(branching) == B and int(depth) == DEPTH
    import time as _time

    in_maps = _make_in_maps(vectors, wl, wr)
    nc = _get_nc()
    last = None
    for attempt in range(6):
        try:
            res = run_bass_kernel_spmd(
                nc, in_maps, core_ids=list(range(NCORES)), trace=trace
            )
            break
        except Exception as e:
            # transient device errors (e.g. NRT_EXEC_UNIT_UNRECOVERABLE left
            # by an interrupted earlier session) clear after a reset cycle,
            # which can take tens of seconds
            last = e
            _time.sleep(5.0 * (attempt + 1))
    else:
        raise last
    return np.asarray(res.results[0]["out"], dtype=np.float32), res


def _make_in_maps(vectors, wl, wr):
    vectors = np.asarray(vectors, dtype=np.float32)
    off = [(B**l - 1) // (B - 1) for l in range(DEPTH + 1)]

    def fm(rows, dt=np.float16):
        return np.ascontiguousarray(rows.T, dtype=dt)

    base = {}
    # W2 = [wl.T ; wr.T] / 3 : agg = W2.T @ [Ul' ; Ur']
    w2 = np.concatenate([np.asarray(wl).T, np.asarray(wr).T], axis=0) / 3.0
    base["w2"] = np.ascontiguousarray(w2, dtype=np.float16)

    # tail x: feature-major packings (col = et*4 + p for xt1; col = t for xt0)
    x1 = vectors[off[1]:off[1] + 4]  # [4 nodes, 512]
    xt1 = np.empty((128, 16), dtype=np.float16)
    for et in range(NT):
        xt1[:, et * 4:(et + 1) * 4] = x1[:, et * 128:(et + 1) * 128].T
    base["xt1"] = xt1
    base["xt0"] = np.ascontiguousarray(
        vectors[off[0]].reshape(NT, 128).T, dtype=np.float16)
    base["identm"] = np.eye(128, dtype=np.float16)
    # combined per-slot weights for the direct tiny-level / root matmuls
    wlT = np.asarray(wl, dtype=np.float32).T
    wrT = np.asarray(wr, dtype=np.float32).T
    base["vw"] = np.ascontiguousarray(np.concatenate(
        [((3 - b) / 3.0) * wlT + (b / 3.0) * wrT for b in range(B)], axis=0
    ), dtype=np.float16)

    # core c owns the two global level-2 subtrees with b-major storage
    # positions {2c, 2c+1}: pos = 4*b + p for global level-2 node g = 4p + b.
    g2 = np.arange(16, dtype=np.int64)
    pos = 4 * (g2 % 4) + (g2 // 4)
    inv2 = np.empty(16, dtype=np.int64)
    inv2[pos] = g2  # global level-2 node at each storage position

    in_maps = []
    for c in range(NCORES):
        roots = inv2[2 * c:2 * c + 2]  # [r=0, r=1] global level-2 nodes
        m = dict(base)
        # level-1 contribution masks: this core's two level-2 nodes share
        # child slot b = c//2 and sit at parent columns (2c)%4, (2c+1)%4
        b = c // 2
        cl = np.zeros(4, dtype=np.float16)
        cr = np.zeros(4, dtype=np.float16)
        for r in range(2):
            p = (2 * c + r) % 4
            cl[p] = 3 - b
            cr[p] = b
        clcr = np.concatenate([cl, cr])  # [8]
        m["clcr"] = np.tile(clcr, (128, NT)).astype(np.float16)
        xr = np.empty((XR_TOT, D), dtype=np.float32)
        for L in range(7):
            gl = L + 2
            n1 = 4**L  # nodes per subtree at this level
            stor = _storage_nodes(L)  # local node at each storage col
            r, q = stor >> (2 * L), stor & (n1 - 1)
            grows = off[gl] + roots[r] * n1 + q  # global row ids, storage order
            if L == 6:
                leaf = vectors[grows]  # [8192, D] leaf vectors, storage order
            elif L == 5:
                x5 = vectors[grows]  # [2048, D]
            else:
                xr[XR_OFF[L]:XR_OFF[L] + NLOC[L]] = vectors[grows]
        # leaf-level weighted sums on the host (fp32, then one f16 rounding)
        ul6 = 3.0 * leaf[0:2048] + 2.0 * leaf[2048:4096] + leaf[4096:6144]
        ur6 = leaf[2048:4096] + 2.0 * leaf[4096:6144] + 3.0 * leaf[6144:8192]
        rec = np.empty((D, REC_TOT), dtype=np.float16)
        for ci, (c0, w) in enumerate(L5_CHUNKS):
            o = REC_OFF[ci]
            rec[:, o:o + w] = ul6[c0:c0 + w].T
            rec[:, o + w:o + 2 * w] = ur6[c0:c0 + w].T
            rec[:, o + 2 * w:o + 3 * w] = x5[c0:c0 + w].T
        m["u6x5"] = rec
        m["xrest"] = fm(xr)
        in_maps.append(m)
    return in_maps


if __name__ == "__main__":
    sys.path.insert(0, "/root/problem")
    d = np.load("/root/problem/ref_cache.npz")
    out = kernel(d["vectors"], d["wl"], d["wr"], 4, 8)
    exp = d["expected"]
    rel = np.linalg.norm(out - exp) / np.linalg.norm(exp)
    print("out[0,:5]:", out[0, :5])
    print("rel:", rel, "absmax:", np.abs(out - exp).max())
